# revision 7
# baseline (speedup 1.0000x reference)
"""CT parallel-beam 2D forward projector on 8 Trainium2 NeuronCores.

v4: W-difference fields + dual-stream gather + v-shifted one-hot binning.

Per view angle the 4 trapezoid tap weights are the difference fields
  W0 = Phi1, W1 = Phi2-Phi1, W2 = Phi3-Phi2, W3 = 1-Phi3
of the cumulative footprint Phi_i = Phic(i-0.5-g), g = f_xi(x)+f_eta(y).
The stacked [512, 4*512] field matrix factors (rank J=64, fp16) per angle;
the device reconstructs pair-interleaved (W0|W1) and (W2|W3) fields with
four 512-col PE matmuls per 128-row chunk.

Device pipeline per (angle, 128-row eta-chunk):
  PE  : psA=(W0|W1), psB=(W2|W3) col-interleaved fields in PSUM fp32
  DVE : pairA words = psA * img2 in ONE fused TT (PSUM fp32 x bf16 -> bf16)
  ACT : psB -> bf16 staging;  DVE: pairB words = staging * img2 (2x mode)
  POOL: ONE indirect_copy moves fp32 words (bf16 field pairs) for BOTH
        pairs through a dual stream: 512 A-slots (first pixel of each
        xi-bin) + 160 B-slots (second pixel of L==2 bins, compacted)
  PE  : 8 binning matmuls, one per (pair-field, A/B region).  The tap
        shift s rides on the PSUM PARTITION axis via v-shifted one-hots
        (lhs = column-sliced views of one [128,102] one-hot buffer), so
        every matmul is a single-piece 512-col (A) or 160-col (B) write.
  ACT : R psum [99, 672] -> SBUF, DMA out
Host: per-(angle,chunk) collapse proj[n] += R[v',m] at n = bxi_min +
beta0 + v' + m (A) / n = ... + l2[j] (B), plus numpy path for the two
degenerate axis-aligned angles.

SPMD: one program for all 8 cores. Cores 0-3 process "class X" angles
(|cos| >= sin) on img; cores 4-7 process "class Y" angles on img.T. All
per-angle variation (SVD factors, one-hots, gather streams) is input data.
"""

import numpy as np

Nx = Ny = 512
Nu = 768
NTHETA = 180
HALF_U = (Nu - 1) / 2.0
NCORES = 8
import os as _os
APC = int(_os.environ.get("CT_APC", "23"))   # angles per core
NCHUNK = 4        # eta chunks of 128
VP = 96           # local v bins per chunk (beta span < 92)
VPX = VP + 3      # v' rows incl. tap shift 0..3
OHW = VPX + 3     # one-hot buffer width (slices [3-f : 3-f+VPX])
AW = 512          # A-region bins (W <= 512 always)
BW = 160          # B-region slots (#L2 <= 150)
PW = AW + BW      # per-pair gather width / R width
MPAD = PW         # gather output words per pair (= 672 = 16*42)
SRCW = 520        # gather source words per pair: fields 0:512, zeros 512:520
ZW = 512          # zero word index
J = int(_os.environ.get("CT_J", "64"))  # SVD rank per angle
B_RECT = 1e-4     # below this min-slope, use the host rect path

_PROGRAM_CACHE = {}


def _np_bf16():
    import ml_dtypes
    return ml_dtypes.bfloat16


# --------------------------------------------------------------------------
# host tables
# --------------------------------------------------------------------------

def _angle_tables(theta_val):
    th = float(theta_val)
    c, s = np.cos(th), np.sin(th)
    ac, asn = abs(c), abs(s)
    A, B = max(ac, asn), min(ac, asn)
    cls = 0 if ac >= asn else 1
    a_xi, a_eta = (c, s) if cls == 0 else (s, c)
    z0 = HALF_U - (ac + asn) / 2 - 255.5 * (c + s)
    grid = np.arange(512)
    pxi = a_xi * grid + z0
    bxi = np.floor(pxi).astype(np.int64)
    fxi = pxi - bxi
    peta = a_eta * grid
    beta = np.floor(peta).astype(np.int64)
    feta = peta - beta
    rect = B <= B_RECT
    return dict(A=A, B=B, cls=cls, rect=rect,
                bxi=bxi, fxi=fxi, beta=beta, feta=feta)


def _gather_tables(T):
    """A-stream first-pixel indices xa and L==2 bin list l2."""
    bxi = T["bxi"]
    bxi_min = int(bxi.min())
    mloc = bxi - bxi_min
    W = int(mloc.max()) + 1
    assert W <= AW
    order = np.argsort(mloc, kind="stable")
    sorted_m = mloc[order]
    first = np.searchsorted(sorted_m, np.arange(W), side="left")
    last = np.searchsorted(sorted_m, np.arange(W), side="right")
    L = last - first
    assert L.min() >= 1 and L.max() <= 2
    xa = order[first]          # first xi of each bin (order is stable)
    l2 = np.where(L == 2)[0]
    assert len(l2) <= BW, len(l2)
    return dict(bxi_min=bxi_min, W=W, xa=xa, l2=l2)


def _wrap_idx(stream):
    """[MPAD] int -> [128, MPAD//16] uint16 wrapped per 16-partition groups."""
    w = stream.reshape(MPAD // 16, 16).T.astype(np.uint16)
    return np.tile(w, (8, 1))


def _phic(t, A, B):
    q = 1.0 / (2 * A * B)
    r = lambda x: np.square(np.maximum(x, 0.0))
    return q * (r(t) - r(t - B) - r(t - A) + r(t - A - B))


def _angle_factors(T):
    """Rank-J factorization of the stacked W-difference fields.

    Returns ByT [J, 512] fp16 and Bx [J, 2048] fp16 with columns permuted
    so device psA cols are (W0|W1) lane-interleaved (xi 0:256 then
    256:512) and psB cols likewise (W2|W3).
    """
    A, B = T["A"], T["B"]
    g = T["feta"][:, None] + T["fxi"][None, :]          # [512, 512]
    P1 = _phic(0.5 - g, A, B)
    P2 = _phic(1.5 - g, A, B)
    P3 = _phic(2.5 - g, A, B)
    M = np.concatenate([P1, P2 - P1, P3 - P2, 1.0 - P3], axis=1)  # [512,2048]
    MMt = M @ M.T
    w, V = np.linalg.eigh(MMt)
    order = np.argsort(w)[::-1][:J]
    w = np.maximum(w[order], 1e-20)
    V = V[:, order]                                     # [512, J]
    s = np.sqrt(np.sqrt(w))                             # sigma^(1/2)
    ByT = (V * s[None, :]).T                            # [J, 512]
    Bx = (V / s[None, :]).T @ M                         # [J, 2048]
    # device col c (global block b = c//512, j = c%512):
    #   xi = (b%2)*256 + j//2, field f = (b//2)*2 + j%2 -> M col f*512+xi
    cg = np.arange(2048)
    b, jj = cg // 512, cg % 512
    perm = ((b // 2) * 2 + jj % 2) * 512 + (b % 2) * 256 + jj // 2
    Bx = Bx[:, perm]
    return ByT.astype(np.float16), Bx.astype(np.float16)


def _core_inputs(img_layout, angle_list, tables):
    """Build the input map for one core. img_layout: [512,512] f32 [eta,xi]."""
    bf16 = _np_bf16()
    A_ = APC
    img = np.ascontiguousarray(img_layout).astype(np.float32)
    imgc = img.reshape(NCHUNK, 128, 512)
    img2 = np.repeat(imgc, 2, axis=2)                   # [4,128,1024]

    ByT_t = np.zeros((A_, J, 512), dtype=np.float16)
    Bx_t = np.zeros((A_, J, 2048), dtype=np.float16)
    oh_t = np.zeros((A_, 128, NCHUNK * OHW), dtype=np.float32)
    idx_t = np.zeros((A_, 128, MPAD // 16), dtype=np.uint16)
    meta = []
    for ai, a in enumerate(angle_list):
        T = tables[a]
        G = _gather_tables(T)
        ByT, Bx = _angle_factors(T)
        ByT_t[ai] = ByT
        Bx_t[ai] = Bx
        beta = T["beta"]
        beta0 = []
        for k in range(NCHUNK):
            sl = slice(k * 128, (k + 1) * 128)
            vloc = beta[sl] - beta[sl].min()
            assert vloc.min() >= 0 and vloc.max() < VP
            oh_t[ai, np.arange(128), k * OHW + 3 + vloc] = 1.0
            beta0.append(int(beta[sl].min()))
        W, xa, l2 = G["W"], G["xa"], G["l2"]
        sA = np.full(AW, ZW, dtype=np.int64)
        sA[:W] = xa[:W]
        sB = np.full(BW, ZW, dtype=np.int64)
        if len(l2):
            sB[:len(l2)] = xa[l2] + 1
        stream = np.concatenate([sA, sB])
        idx_t[ai] = _wrap_idx(stream)
        meta.append(dict(angle=a, bxi_min=G["bxi_min"], W=W, l2=l2,
                         beta0=beta0))
    in_map = {
        "img2_t": img2.astype(bf16),
        "ByT_t": ByT_t,
        "Bx_t": Bx_t,
        "oh_t": oh_t.astype(bf16),
        "idx_t": idx_t,
    }
    return in_map, meta


# --------------------------------------------------------------------------
# the bass program (identical for all cores)
# --------------------------------------------------------------------------

def _build_program():
    if "nc" in _PROGRAM_CACHE:
        return _PROGRAM_CACHE["nc"]

    import concourse.bass as bass
    import concourse.tile as tile
    from concourse import bacc, mybir
    from contextlib import ExitStack

    dt = mybir.dt
    ALU = mybir.AluOpType

    nc = bacc.Bacc("TRN2", target_bir_lowering=False, debug=False,
                   num_devices=NCORES)

    img2_t = nc.dram_tensor("img2_t", [NCHUNK, 128, 1024], dt.bfloat16,
                            kind="ExternalInput").ap()
    ByT_t = nc.dram_tensor("ByT_t", [APC, J, 512], dt.float16,
                           kind="ExternalInput").ap()
    Bx_t = nc.dram_tensor("Bx_t", [APC, J, 2048], dt.float16,
                          kind="ExternalInput").ap()
    oh_t = nc.dram_tensor("oh_t", [APC, 128, NCHUNK * OHW], dt.bfloat16,
                          kind="ExternalInput").ap()
    idx_t = nc.dram_tensor("idx_t", [APC, 128, MPAD // 16], dt.uint16,
                           kind="ExternalInput").ap()
    r_out = nc.dram_tensor("r_out", [APC, NCHUNK, VPX, PW], dt.float32,
                           kind="ExternalOutput").ap()

    with tile.TileContext(nc) as tc, ExitStack() as ctx:
        img_pool = ctx.enter_context(tc.tile_pool(name="img", bufs=1))
        src_pool = ctx.enter_context(tc.tile_pool(name="src", bufs=1))
        stg_pool = ctx.enter_context(tc.tile_pool(name="stg", bufs=1))
        tab_pool = ctx.enter_context(tc.tile_pool(name="tabs", bufs=2))
        g_pool = ctx.enter_context(tc.tile_pool(name="gath", bufs=1))
        o_pool = ctx.enter_context(tc.tile_pool(name="outs", bufs=1))
        psa_pool = ctx.enter_context(tc.tile_pool(name="psumA", bufs=2,
                                                  space="PSUM"))
        psb_pool = ctx.enter_context(tc.tile_pool(name="psumB", bufs=1,
                                                  space="PSUM"))
        psr_pool = ctx.enter_context(tc.tile_pool(name="psumR", bufs=1,
                                                  space="PSUM"))

        img2_ch, srcs = [], []
        for k in range(NCHUNK):
            t = img_pool.tile([128, 1024], dt.bfloat16, tag=f"img2c{k}")
            nc.sync.dma_start(t[:], img2_t[k])
            img2_ch.append(t)
            sr = src_pool.tile([128, 4 * SRCW], dt.bfloat16, tag=f"src{k}")
            nc.vector.memset(sr[:, 1024:1040], 0.0)
            nc.vector.memset(sr[:, 2064:2080], 0.0)
            srcs.append(sr)

        tabs = {}

        def load_tabs(ai):
            idxt = tab_pool.tile([128, MPAD // 16], dt.uint16, tag="idx")
            nc.sync.dma_start(idxt[:], idx_t[ai])
            bx = tab_pool.tile([J, 2048], dt.float16, tag="bx")
            nc.sync.dma_start(bx[:], Bx_t[ai])
            byt = tab_pool.tile([J, 512], dt.float16, tag="byt")
            nc.sync.dma_start(byt[:], ByT_t[ai])
            oht = tab_pool.tile([128, NCHUNK * OHW], dt.bfloat16, tag="oh")
            nc.sync.dma_start(oht[:], oh_t[ai])
            tabs[ai] = (idxt, bx, byt, oht)

        def front(ai, k):
            idxt, bx, byt, _ = tabs[ai]
            byk = byt[:, 128 * k:128 * (k + 1)]
            src = srcs[k]

            psA = psa_pool.tile([128, 1024], dt.float32, tag="psA")
            nc.tensor.matmul(psA[:, 0:512], byk, bx[:, 0:512],
                             start=True, stop=True)
            nc.tensor.matmul(psA[:, 512:1024], byk, bx[:, 512:1024],
                             start=True, stop=True)
            psB = psb_pool.tile([128, 1024], dt.float32, tag="psB")
            nc.tensor.matmul(psB[:, 0:512], byk, bx[:, 1024:1536],
                             start=True, stop=True)
            nc.tensor.matmul(psB[:, 512:1024], byk, bx[:, 1536:2048],
                             start=True, stop=True)

            # pair A: fused PSUM*img -> bf16 on DVE
            nc.vector.tensor_tensor(src[:, 0:1024], psA[:],
                                    img2_ch[k][:], ALU.mult)
            # pair B: ACT drain to bf16, then DVE 2x mult
            stg = stg_pool.tile([128, 1024], dt.bfloat16, tag=f"stg{k}")
            nc.scalar.copy(stg[:], psB[:])
            nc.vector.tensor_tensor(src[:, 1040:2064], stg[:],
                                    img2_ch[k][:], ALU.mult)

            gtA = g_pool.tile([128, MPAD], dt.float32, tag=f"gtA{k}")
            nc.gpsimd.indirect_copy(
                gtA[:], src[:, 0:2 * SRCW].bitcast(dt.float32),
                idxt[:], True)
            gtB = g_pool.tile([128, MPAD], dt.float32, tag=f"gtB{k}")
            nc.gpsimd.indirect_copy(
                gtB[:], src[:, 2 * SRCW:4 * SRCW].bitcast(dt.float32),
                idxt[:], True)
            return gtA, gtB

        def back(ai, k, gts):
            gtA, gtB = gts
            _, _, _, oht = tabs[ai]
            vA = gtA[:].bitcast(dt.bfloat16).rearrange(
                "p (w l) -> p w l", l=2)
            vB = gtB[:].bitcast(dt.bfloat16).rearrange(
                "p (w l) -> p w l", l=2)
            lanesA = [vA[:, 0:AW, 0], vA[:, 0:AW, 1],
                      vB[:, 0:AW, 0], vB[:, 0:AW, 1]]
            lanesB = [vA[:, AW:PW, 0], vA[:, AW:PW, 1],
                      vB[:, AW:PW, 0], vB[:, AW:PW, 1]]

            ps = psr_pool.tile([VPX, PW], dt.float32, tag="ps")
            for f in range(4):
                lhs = oht[:, k * OHW + 3 - f: k * OHW + 3 - f + VPX]
                nc.tensor.matmul(ps[:, 0:AW], lhs, lanesA[f],
                                 start=(f == 0), stop=(f == 3))
                nc.tensor.matmul(ps[:, AW:PW], lhs, lanesB[f],
                                 start=(f == 0), stop=(f == 3))

            rout = o_pool.tile([VPX, PW], dt.float32, tag=f"rout{k}")
            nc.scalar.copy(rout[:], ps[:])
            nc.scalar.dma_start(r_out[ai, k], rout[:])

        # software pipeline: back-half of slot i-LAG runs alongside the
        # front-half of slot i so PE never waits on the current gather
        LAG = 2
        slots = [(ai, k) for ai in range(APC) for k in range(NCHUNK)]
        load_tabs(0)
        pend = {}
        for i, (ai, k) in enumerate(slots):
            if k == 0 and ai + 1 < APC:
                load_tabs(ai + 1)
            pend[i] = (ai, k, front(ai, k))
            j = i - LAG
            if j in pend:
                aj, kj, gts = pend.pop(j)
                back(aj, kj, gts)
        for j in sorted(pend):
            aj, kj, gts = pend.pop(j)
            back(aj, kj, gts)

    nc.compile()
    _PROGRAM_CACHE["nc"] = nc
    return nc


# --------------------------------------------------------------------------
# host-side rect path (degenerate angles) — numpy port of the reference
# --------------------------------------------------------------------------

def _host_project(img, theta_vals):
    y = (np.arange(Ny) - (Ny - 1) / 2.0)
    x = (np.arange(Nx) - (Nx - 1) / 2.0)
    y2d, x2d = np.meshgrid(y, x, indexing="ij")
    img_v = img.reshape(-1).astype(np.float64)
    out = np.zeros((len(theta_vals), Nu), dtype=np.float64)
    K = 4
    for t, th in enumerate(theta_vals):
        th = float(th)
        cos_t, sin_t = np.cos(th), np.sin(th)
        ac, asn = abs(cos_t), abs(sin_t)
        h = min(1.0 / ac if ac > 0 else np.inf, 1.0 / asn if asn > 0 else np.inf)
        b1 = abs(asn - ac)
        b2 = abs(asn + ac)
        u0 = x2d * cos_t + y2d * sin_t
        u1 = u0 - b2 / 2
        u2 = u0 - b1 / 2
        u3 = u0 + b1 / 2
        u4 = u0 + b2 / 2
        base = np.floor(u1 + HALF_U).astype(np.int64)
        den12 = (u2 - u1) + (u1 == u2)
        den34 = (u4 - u3) + (u3 == u4)
        acc = np.zeros(Nu + 8, dtype=np.float64)
        for k in range(K):
            idx = base + k
            u = idx - HALF_U
            lo, hi = u - 0.5, u + 0.5
            uA = np.maximum(u1, lo); uB = np.minimum(u2, hi)
            w = (uB > uA) * (h / (2.0 * den12)) * ((uB - u1) ** 2 - (uA - u1) ** 2)
            uA = np.maximum(u2, lo); uB = np.minimum(u3, hi)
            w = w + (uB > uA) * h * (uB - uA)
            uA = np.maximum(u3, lo); uB = np.minimum(u4, hi)
            w = w + (uB > uA) * (h / (2.0 * den34)) * ((uA - u4) ** 2 - (uB - u4) ** 2)
            np.add.at(acc, np.clip(idx.reshape(-1), 0, Nu - 1),
                      img_v * w.reshape(-1))
        out[t] = acc[:Nu]
    return out.astype(np.float32)


# --------------------------------------------------------------------------
# main entry
# --------------------------------------------------------------------------

def kernel(img, theta):
    img = np.asarray(img, dtype=np.float32)
    theta = np.asarray(theta, dtype=np.float32)
    assert img.shape == (Ny, Nx) and theta.shape == (NTHETA,)

    tables = {a: _angle_tables(theta[a]) for a in range(NTHETA)}
    rect_angles = [a for a in range(NTHETA) if tables[a]["rect"]]
    dev_angles = [a for a in range(NTHETA) if not tables[a]["rect"]]
    clsX = [a for a in dev_angles if tables[a]["cls"] == 0]
    clsY = [a for a in dev_angles if tables[a]["cls"] == 1]
    assert len(clsX) <= 4 * APC and len(clsY) <= 4 * APC

    def assign(lst, ncores):
        groups = [lst[i::ncores] for i in range(ncores)]
        return [g + [g[-1]] * (APC - len(g)) if g else [dev_angles[0]] * APC
                for g in groups]

    core_angles = assign(clsX, 4) + assign(clsY, 4)

    imgT = np.ascontiguousarray(img.T)
    in_maps, metas = [], []
    for ci in range(NCORES):
        layout = img if ci < 4 else imgT
        im, meta = _core_inputs(layout, core_angles[ci], tables)
        in_maps.append(im)
        metas.append(meta)

    nc = _build_program()
    from concourse import bass_utils
    import os
    trace = bool(int(os.environ.get("CT_TRACE", "0")))
    res = bass_utils.run_bass_kernel_spmd(nc, in_maps,
                                          core_ids=list(range(NCORES)),
                                          trace=trace)
    _PROGRAM_CACHE["exec_time_ns"] = getattr(res, "exec_time_ns", None)
    _PROGRAM_CACHE["last_results"] = res

    OFF = 128
    acc = np.zeros(OFF + Nu + OFF + AW, dtype=np.float64)
    proj = np.zeros((NTHETA, Nu), dtype=np.float64)
    vrows = np.arange(VPX)
    done = set()
    for ci in range(NCORES):
        R = res.results[ci]["r_out"]  # [APC, NCHUNK, VPX, PW]
        for ai, m in enumerate(metas[ci]):
            a = m["angle"]
            if a in done:
                continue
            done.add(a)
            acc[:] = 0.0
            l2 = m["l2"]
            nB = len(l2)
            for k in range(NCHUNK):
                n0 = OFF + m["bxi_min"] + m["beta0"][k]
                Rk = R[ai, k].astype(np.float64)
                for vq in range(VPX):
                    acc[n0 + vq: n0 + vq + AW] += Rk[vq, :AW]
                if nB:
                    np.add.at(acc, (n0 + vrows)[:, None] + l2[None, :],
                              Rk[:, AW:AW + nB])
            proj[a] = acc[OFF:OFF + Nu]

    if rect_angles:
        proj[rect_angles] = _host_project(img, theta[rect_angles])
    return proj.astype(np.float32)
